# revision 1
# baseline (speedup 1.0000x reference)
"""Trainium2 Bass kernel for nn_BaseSearchBasedModel (sparse attention).

Math restructuring (exact up to fp32 rounding):
  topk   = user_seq_emb[b, indices[b,k]]                      (device gather)
  q      = tgt @ WQ[h] + bQ[h]
  scores = q . (topk @ WK[h] + bK[h]) / 8
         = topk . (A[h]^T tgt + c[h]) / 8  + const(b,h)       A = WQ WK^T, c = WK bQ
    The const(b,h) term (q.bK) is constant over the softmax axis -> drops out.
  heads  = softmax(scores) @ topk @ WV[h]                     (WV folded after softmax)
  mhta   = sum_h ctx[h] @ G[h] + bias0                        G = WV[h] WO_h,
                                                              bias0 = sum_h bV[h] WO_h + bO
  logit  = MLP(concat(mhta, tgt))

Sharding: pure data parallel, batch 2048 -> 8 cores x 256.
Per core: indices are turned into flat row offsets on device (cast/transpose/iota/add),
8 indirect-DMA gathers of 32 samples each land as [j=128, (sample, d)] tiles; per pair
of samples one PE transpose + one block-diagonal scores matmul; softmax via ACT exp +
PE ones-matmul column sums + DVE reciprocal; context and output head via folded
64x64 matrices."""

import sys

if "/opt/trn_rl_repo" not in sys.path:
    sys.path.insert(0, "/opt/trn_rl_repo")

import numpy as np

import concourse.bass as bass
import concourse.tile as tile
import concourse.mybir as mybir
from concourse import bacc
from concourse.bass_utils import run_bass_kernel_spmd
from concourse.masks import make_identity

F32 = mybir.dt.float32
I32 = mybir.dt.int32
I16 = mybir.dt.int16
AF = mybir.ActivationFunctionType

B, L, K, D, H = 2048, 1024, 128, 64, 4
N_CORES = 8
B_LOC = B // N_CORES  # 256


def build(b_loc=B_LOC):
    """Build the per-core Bass module. b_loc must be a multiple of 32."""
    assert b_loc % 32 == 0
    n_grp = b_loc // 32            # gather groups of 32 samples
    n_pair = b_loc // 2            # sample pairs
    n_bt = (b_loc + 127) // 128    # 128-row tiles over the local batch

    nc = bacc.Bacc("TRN2", target_bir_lowering=False, debug=False, num_devices=N_CORES)

    useq = nc.dram_tensor("useq", [b_loc * L, D], F32, kind="ExternalInput").ap()
    tgt = nc.dram_tensor("tgt", [b_loc, D], F32, kind="ExternalInput").ap()
    idx = nc.dram_tensor("idx", [b_loc, K], I32, kind="ExternalInput").ap()
    wq = nc.dram_tensor("wq", [H * D, D], F32, kind="ExternalInput").ap()
    wk = nc.dram_tensor("wk", [H * D, D], F32, kind="ExternalInput").ap()
    wv = nc.dram_tensor("wv", [H * D, D], F32, kind="ExternalInput").ap()
    bq = nc.dram_tensor("bq", [H, D], F32, kind="ExternalInput").ap()
    bv = nc.dram_tensor("bv", [H, D], F32, kind="ExternalInput").ap()
    wo = nc.dram_tensor("wo", [H * D, D], F32, kind="ExternalInput").ap()
    bo = nc.dram_tensor("bo", [D, 1], F32, kind="ExternalInput").ap()
    w1 = nc.dram_tensor("w1", [2 * D, D], F32, kind="ExternalInput").ap()
    b1 = nc.dram_tensor("b1", [D, 1], F32, kind="ExternalInput").ap()
    w2 = nc.dram_tensor("w2", [D, 1], F32, kind="ExternalInput").ap()
    b2 = nc.dram_tensor("b2", [1, 1], F32, kind="ExternalInput").ap()
    logit = nc.dram_tensor("logit", [b_loc, 1], F32, kind="ExternalOutput").ap()

    with tile.TileContext(nc) as tc, \
         tc.tile_pool(name="const", bufs=1) as const, \
         tc.tile_pool(name="stage", bufs=3) as stage, \
         tc.tile_pool(name="gath", bufs=4) as gath_pool, \
         tc.tile_pool(name="ptsb", bufs=4) as pt_pool, \
         tc.tile_pool(name="small", bufs=2) as small, \
         tc.tile_pool(name="expsb", bufs=2) as exp_pool, \
         tc.tile_pool(name="gidx", bufs=8) as gidx_pool, \
         tc.tile_pool(name="ps_a", bufs=1, space="PSUM") as ps_a, \
         tc.tile_pool(name="ptps", bufs=2, space="PSUM") as ptps_pool, \
         tc.tile_pool(name="scx", bufs=2, space="PSUM") as scx_pool, \
         tc.tile_pool(name="zc", bufs=1, space="PSUM") as zc_pool, \
         tc.tile_pool(name="mhps", bufs=1, space="PSUM") as mh_pool:

        ident = const.tile([128, 128], F32, tag="ident")
        make_identity(nc, ident[:])
        ones_col = const.tile([128, 1], F32, tag="ones_col")
        nc.vector.memset(ones_col[:], 1.0)
        ones_row = const.tile([1, 128], F32, tag="ones_row")
        nc.vector.memset(ones_row[:], 1.0)

        # ---- index prep for dma_gather (first: gathers depend only on this) ----
        # Gather order i = 32*128*g + 128*s + j; the SWDGE ucode reads idx i at
        # idxs[i % 16, i // 16] (int16), replicated into all 8 16-partition
        # blocks (one per Q7 core). Flat i-order == row-major dram order of
        # idx[s, j], so the wrapped layout is one strided load; values are row
        # offsets local to the group's 32-sample slice: idx[s,j] + 1024*(s%32).
        add16 = const.tile([16, 256], I16, tag="add16")
        nc.gpsimd.iota(add16[:], pattern=[[L, 32], [0, 8]], base=0,
                       channel_multiplier=0)
        # fast-start tile covering the first 16 samples (chunk 0)
        w16a = const.tile([128, 128], I16, tag="w16a")
        rawa = small.tile([16, 128], I16, tag="rawa")
        lo16a = idx[0:16, :].bitcast(I16).rearrange(
            "s (j two) -> s j two", two=2)[:, :, 0]
        nc.scalar.dma_start(out=rawa[:],
                            in_=lo16a.rearrange("s (cj p) -> p (s cj)", p=16))
        nc.vector.tensor_tensor(out=w16a[0:16, :], in0=rawa[:], in1=add16[:, 0:128],
                                op=mybir.AluOpType.add)
        for r in range(1, 8):
            nc.scalar.dma_start(out=w16a[16 * r:16 * (r + 1), :], in_=w16a[0:16, :])
        w16s = []
        for g in range(n_grp):
            # idx values fit int16: read just the low halves of the int32s
            rawg = small.tile([16, 256], I16, tag="rawg")
            lo16 = idx[32 * g:32 * (g + 1), :].bitcast(I16).rearrange(
                "s (j two) -> s j two", two=2)[:, :, 0]
            nc.scalar.dma_start(
                out=rawg[:],
                in_=lo16.rearrange("s (cj p) -> p (s cj)", p=16))
            w16 = gidx_pool.tile([128, 256], I16, tag="w16")
            nc.vector.tensor_tensor(out=w16[0:16, :], in0=rawg[:], in1=add16[:],
                                    op=mybir.AluOpType.add)
            for r in range(1, 8):
                nc.scalar.dma_start(out=w16[16 * r:16 * (r + 1), :], in_=w16[0:16, :])
            w16s.append(w16)

        # ---- weight transposes: wqT/wkT/wvT [64, 256] = [e, (h, d)] ----
        wT = {}
        for name, dram in (("wq", wq), ("wk", wk), ("wv", wv)):
            t_sb = const.tile([D, H * D], F32, tag=f"{name}T")
            for t in range(2):
                s = stage.tile([128, D], F32, tag="wstage")
                nc.sync.dma_start(out=s[:], in_=dram[128 * t:128 * (t + 1), :])
                tr = ps_a.tile([D, 128], F32, tag="pa")
                nc.tensor.transpose(tr[:], s[:], ident[:])
                nc.scalar.activation(t_sb[:, 128 * t:128 * (t + 1)], tr[:], AF.Copy)
            wT[name] = t_sb

        # ---- wo_r [64, 256] = [e, (h, f)] (reshaped, not transposed) ----
        wo_r = const.tile([D, H * D], F32, tag="wo_r")
        nc.sync.dma_start(out=wo_r[:].rearrange("p (h f) -> p h f", h=H),
                          in_=wo[:].rearrange("(h e) f -> e h f", h=H))

        # ---- small bias columns ----
        bqcol = const.tile([D, H], F32, tag="bqcol")
        nc.sync.dma_start(out=bqcol[:], in_=bq[:].rearrange("h e -> e h"))
        bvcol = const.tile([D, H], F32, tag="bvcol")
        nc.sync.dma_start(out=bvcol[:], in_=bv[:].rearrange("h e -> e h"))
        bocol = const.tile([D, 1], F32, tag="bocol")
        nc.sync.dma_start(out=bocol[:], in_=bo[:])
        b1col = const.tile([D, 1], F32, tag="b1col")
        nc.sync.dma_start(out=b1col[:], in_=b1[:])
        w1_sb = const.tile([2 * D, D], F32, tag="w1")
        nc.sync.dma_start(out=w1_sb[:], in_=w1[:])
        w2_sb = const.tile([D, 1], F32, tag="w2")
        nc.sync.dma_start(out=w2_sb[:], in_=w2[:])
        b2_sb = const.tile([1, 1], F32, tag="b2")
        nc.sync.dma_start(out=b2_sb[:], in_=b2[:])

        # ---- target transpose: tgtT [64, b_loc]; also xT rows 64:128 ----
        tgtT = const.tile([D, b_loc], F32, tag="tgtT")
        xT = const.tile([128, b_loc], F32, tag="xT")
        for t in range(n_bt):
            r0, r1 = 128 * t, min(128 * (t + 1), b_loc)
            n = r1 - r0
            s = stage.tile([128, D], F32, tag="tstage")
            nc.sync.dma_start(out=s[:n, :], in_=tgt[r0:r1, :])
            tr = ps_a.tile([D, 128], F32, tag="pa")
            nc.tensor.transpose(tr[:, :n], s[:n, :], ident[:n, :n])
            nc.scalar.activation(tgtT[:, r0:r1], tr[:, :n], AF.Copy)
            nc.scalar.activation(xT[D:2 * D, r0:r1], tr[:, :n], AF.Copy)

        # ---- index prep for dma_gather ----
        # Global gather order i = 32*128*g + 128*s + j; the SWDGE ucode reads
        # idx i at idxs[i % 16, i // 16] (int16), replicated into all 8
        # 16-partition blocks (one per Q7 core). Since flat i-order equals the
        # row-major dram order of idx[s, j], the wrapped layout is one strided
        # load. Values are row offsets local to the group's 32-sample slice of
        # useq: idx[s, j] + 1024*(s % 32).

        # ---- per-head folded matrices ----
        # A_sb[:, 64h:64h+64] = (WQ[h] @ WK[h]^T) / 8 ; c_col = (WK[h] @ bQ[h]) / 8
        A_sb = const.tile([D, H * D], F32, tag="A")
        c_col = const.tile([128, H], F32, tag="c_col")
        c_ps = ps_a.tile([128, H], F32, tag="pa")
        for h in range(H):
            a_ps = ps_a.tile([D, D], F32, tag="pa")
            nc.tensor.matmul(a_ps[:], lhsT=wT["wq"][:, D * h:D * (h + 1)],
                             rhs=wT["wk"][:, D * h:D * (h + 1)], start=True, stop=True)
            nc.scalar.activation(A_sb[:, D * h:D * (h + 1)], a_ps[:], AF.Copy,
                                 scale=0.125)
            nc.tensor.matmul(c_ps[0:D, h:h + 1], lhsT=wT["wk"][:, D * h:D * (h + 1)],
                             rhs=bqcol[:, h:h + 1], start=True, stop=True)
            nc.tensor.matmul(c_ps[D:2 * D, h:h + 1], lhsT=wT["wk"][:, D * h:D * (h + 1)],
                             rhs=bqcol[:, h:h + 1], start=True, stop=True)
        nc.scalar.activation(c_col[:], c_ps[:], AF.Copy, scale=0.125)

        # ---- qk block-diagonal tile [128, 8*n_pair] ----
        # pair q columns 8q..8q+7: cols 0-3 = even sample heads (rows 0:64),
        # cols 4-7 = odd sample heads (rows 64:128); rest zero.
        qk_bd = const.tile([128, 8 * n_pair], F32, tag="qk_bd")
        nc.vector.memset(qk_bd[:], 0.0)
        tgtT_v = tgtT[:].rearrange("p (s two) -> p s two", two=2)
        qk_v = qk_bd[:].rearrange("p (q c) -> p q c", c=8)
        for h in range(H):
            qk_ps = ps_a.tile([128, n_pair], F32, tag="pa")
            nc.tensor.matmul(qk_ps[0:D, :], lhsT=A_sb[:, D * h:D * (h + 1)],
                             rhs=tgtT_v[:, :, 0], start=True, stop=True)
            nc.tensor.matmul(qk_ps[D:2 * D, :], lhsT=A_sb[:, D * h:D * (h + 1)],
                             rhs=tgtT_v[:, :, 1], start=True, stop=True)
            nc.scalar.activation(qk_v[0:D, :, h], qk_ps[0:D, :], AF.Identity,
                                 bias=c_col[0:D, h:h + 1], scale=1.0)
            nc.scalar.activation(qk_v[D:2 * D, :, 4 + h], qk_ps[D:2 * D, :], AF.Identity,
                                 bias=c_col[D:2 * D, h:h + 1], scale=1.0)

        # ---- G_sb [128, 256]: rows 0:64 and 64:128 both hold G[h] = WV[h] @ WO_h ----
        G_sb = const.tile([128, H * D], F32, tag="G")
        for h in range(H):
            g_ps = ps_a.tile([128, D], F32, tag="pa")
            nc.tensor.matmul(g_ps[0:D, :], lhsT=wT["wv"][:, D * h:D * (h + 1)],
                             rhs=wo_r[:, D * h:D * (h + 1)], start=True, stop=True)
            nc.tensor.matmul(g_ps[D:2 * D, :], lhsT=wT["wv"][:, D * h:D * (h + 1)],
                             rhs=wo_r[:, D * h:D * (h + 1)], start=True, stop=True)
            nc.scalar.activation(G_sb[:, D * h:D * (h + 1)], g_ps[:], AF.Copy)

        # ---- bias0 = sum_h bV[h] @ WO_h + bO, as a [64, 1] column ----
        b0_ps = ps_a.tile([1, D], F32, tag="pa")
        for h in range(H):
            nc.tensor.matmul(b0_ps[:], lhsT=bvcol[:, h:h + 1],
                             rhs=wo_r[:, D * h:D * (h + 1)],
                             start=(h == 0), stop=(h == H - 1))
        b0row = stage.tile([1, D], F32, tag="b0row")
        nc.scalar.activation(b0row[:], b0_ps[:], AF.Copy)
        b0c_ps = ps_a.tile([D, 1], F32, tag="pa")
        nc.tensor.transpose(b0c_ps[:], b0row[:], ident[0:1, 0:1])
        bias0 = const.tile([D, 1], F32, tag="bias0")
        nc.vector.tensor_tensor(out=bias0[:], in0=b0c_ps[:], in1=bocol[:],
                                op=mybir.AluOpType.add)

        # ---- main loop over chunks of 16 samples ----
        n_chunk = b_loc // 16
        ctxn = const.tile([128, 2 * b_loc], F32, tag="ctxn")
        ctxn_v = ctxn[:].rearrange("p (cc q c) -> p cc q c", cc=n_chunk, c=4)
        mh_e = mh_pool.tile([D, n_pair], F32, tag="mh_e")
        mh_o = mh_pool.tile([D, n_pair], F32, tag="mh_o")
        for c in range(n_chunk):
            g, half = c // 2, c % 2
            gth = gath_pool.tile([128, 16 * D], F32, tag="gath")
            idxs_c = w16a[:] if c == 0 else w16s[g][:, 128 * half:128 * (half + 1)]
            nc.gpsimd.dma_gather(
                out_ap=gth[:].rearrange("p (s d) -> p s d", d=D),
                in_ap=useq[32 * L * g:32 * L * (g + 1), :],
                idxs_ap=idxs_c,
                num_idxs=16 * K, num_idxs_reg=16 * K, elem_size=D,
                single_packet=False)
            sc_ps = scx_pool.tile([128, 64], F32, tag="scx")
            for q in range(8):
                tr_ps = ptps_pool.tile([128, 128], F32, tag="trp")
                nc.tensor.transpose(tr_ps[:], gth[:, 128 * q:128 * (q + 1)], ident[:])
                pT = pt_pool.tile([128, 128], F32, tag="pt")
                nc.vector.tensor_copy(out=pT[:], in_=tr_ps[:])
                Q = 8 * c + q
                nc.tensor.matmul(sc_ps[:, 8 * q:8 * (q + 1)], lhsT=pT[:],
                                 rhs=qk_bd[:, 8 * Q:8 * (Q + 1)], start=True, stop=True)
            exp_sb = exp_pool.tile([128, 64], F32, tag="exp")
            nc.scalar.activation(exp_sb[:], sc_ps[:], AF.Exp)
            # op2 on UNNORMALIZED exp (releases gth asap); 1/Z folded in below.
            ctx_ps = scx_pool.tile([128, 64], F32, tag="scx")
            for q in range(8):
                nc.tensor.matmul(ctx_ps[:, 8 * q:8 * (q + 1)],
                                 lhsT=gth[:, 128 * q:128 * (q + 1)],
                                 rhs=exp_sb[:, 8 * q:8 * (q + 1)], start=True, stop=True)
            z_ps = zc_pool.tile([64, 1], F32, tag="zc")
            nc.tensor.matmul(z_ps[:], lhsT=exp_sb[:], rhs=ones_col[:],
                             start=True, stop=True)
            rz = small.tile([64, 1], F32, tag="rz")
            nc.vector.reciprocal(rz[:], z_ps[:])
            rzr_ps = zc_pool.tile([1, 64], F32, tag="zc")
            nc.tensor.transpose(rzr_ps[:], rz[:], ident[0:64, 0:64])
            rzr = small.tile([1, 64], F32, tag="rzr")
            nc.scalar.activation(rzr[:], rzr_ps[:], AF.Copy)
            rzb_ps = zc_pool.tile([128, 64], F32, tag="zc")
            nc.tensor.matmul(rzb_ps[:], lhsT=ones_row[:], rhs=rzr[:],
                             start=True, stop=True)
            rzb = exp_pool.tile([128, 64], F32, tag="rzb")
            nc.scalar.activation(rzb[:], rzb_ps[:], AF.Copy)
            ctx_v = ctx_ps[:].rearrange("p (q c) -> p q c", c=8)
            rzb_v = rzb[:].rearrange("p (q c) -> p q c", c=8)
            nc.vector.tensor_tensor(out=ctxn_v[0:D, c, :, :], in0=ctx_v[0:D, :, 0:4],
                                    in1=rzb_v[0:D, :, 0:4], op=mybir.AluOpType.mult)
            nc.vector.tensor_tensor(out=ctxn_v[D:2 * D, c, :, :],
                                    in0=ctx_v[D:2 * D, :, 4:8],
                                    in1=rzb_v[D:2 * D, :, 4:8],
                                    op=mybir.AluOpType.mult)
            for h in range(H):
                nc.tensor.matmul(mh_e[:, 8 * c:8 * (c + 1)],
                                 lhsT=G_sb[0:D, D * h:D * (h + 1)],
                                 rhs=ctxn_v[0:D, c, :, h],
                                 start=(h == 0), stop=(h == H - 1))
            for h in range(H):
                nc.tensor.matmul(mh_o[:, 8 * c:8 * (c + 1)],
                                 lhsT=G_sb[D:2 * D, D * h:D * (h + 1)],
                                 rhs=ctxn_v[D:2 * D, c, :, h],
                                 start=(h == 0), stop=(h == H - 1))

        # ---- output head tail: bias + MLP ----
        x_v = xT[:].rearrange("p (s two) -> p s two", two=2)
        nc.scalar.activation(x_v[0:D, :, 0], mh_e[:], AF.Identity, bias=bias0[:],
                             scale=1.0)
        nc.scalar.activation(x_v[0:D, :, 1], mh_o[:], AF.Identity, bias=bias0[:],
                             scale=1.0)
        h1_ps = ps_a.tile([D, b_loc], F32, tag="pa")
        nc.tensor.matmul(h1_ps[:], lhsT=w1_sb[:], rhs=xT[:], start=True, stop=True)
        h1_sb = const.tile([D, b_loc], F32, tag="h1")
        nc.scalar.activation(h1_sb[:], h1_ps[:], AF.Relu, bias=b1col[:], scale=1.0)
        lg_ps = ps_a.tile([1, b_loc], F32, tag="pa")
        nc.tensor.matmul(lg_ps[:], lhsT=w2_sb[:], rhs=h1_sb[:], start=True, stop=True)
        lg_sb = const.tile([1, b_loc], F32, tag="lg")
        nc.scalar.activation(lg_sb[:], lg_ps[:], AF.Identity, bias=b2_sb[:], scale=1.0)
        nc.sync.dma_start(out=logit[:], in_=lg_sb[:])

    nc.compile()
    return nc


def make_in_maps(inputs, b_loc=B_LOC, n_cores=N_CORES):
    """Shard full inputs into per-core in_maps (data parallel over batch)."""
    idx = np.asarray(inputs["indices"])
    if idx.dtype != np.int32:
        idx = idx.astype(np.int32)
    useq = np.ascontiguousarray(np.asarray(inputs["user_seq_emb"], dtype=np.float32))
    tgt = np.ascontiguousarray(np.asarray(inputs["target_emb"], dtype=np.float32)[:, 0, :])
    shared = {
        "wq": np.ascontiguousarray(np.asarray(inputs["WQ"], np.float32).reshape(H * D, D)),
        "wk": np.ascontiguousarray(np.asarray(inputs["WK"], np.float32).reshape(H * D, D)),
        "wv": np.ascontiguousarray(np.asarray(inputs["WV"], np.float32).reshape(H * D, D)),
        "bq": np.ascontiguousarray(np.asarray(inputs["bQ"], np.float32)),
        "bv": np.ascontiguousarray(np.asarray(inputs["bV"], np.float32)),
        "wo": np.ascontiguousarray(np.asarray(inputs["WO"], np.float32)),
        "bo": np.asarray(inputs["bO"], np.float32).reshape(D, 1).copy(),
        "w1": np.ascontiguousarray(np.asarray(inputs["W1"], np.float32)),
        "b1": np.asarray(inputs["b1"], np.float32).reshape(D, 1).copy(),
        "w2": np.ascontiguousarray(np.asarray(inputs["W2"], np.float32)),
        "b2": np.asarray(inputs["b2"], np.float32).reshape(1, 1).copy(),
    }
    in_maps = []
    for c in range(n_cores):
        s = slice(c * b_loc, (c + 1) * b_loc)
        m = dict(shared)
        m["useq"] = useq[s].reshape(b_loc * L, D)
        m["tgt"] = tgt[s]
        m["idx"] = np.ascontiguousarray(idx[s])
        in_maps.append(m)
    return in_maps


_NC_CACHE = {}


def kernel(**inputs):
    if B_LOC not in _NC_CACHE:
        _NC_CACHE[B_LOC] = build(B_LOC)
    nc = _NC_CACHE[B_LOC]
    in_maps = make_in_maps(inputs)
    res = run_bass_kernel_spmd(nc, in_maps, core_ids=list(range(N_CORES)))
    return np.concatenate([res.results[c]["logit"] for c in range(N_CORES)], axis=0)



# revision 11
# speedup vs baseline: 2.6909x; 2.6909x over previous
"""Trainium2 Bass kernel for nn_BaseSearchBasedModel (sparse attention).

Math restructuring (exact up to rounding):
  topk   = user_seq_emb[b, indices[b,k]]                      (device gather)
  scores = topk . (A[h]^T tgt + c[h]) / 8  + const(b,h)       A = WQ WK^T, c = WK bQ
    The const(b,h) term (q.bK) is constant over the softmax axis -> drops out.
  heads  = softmax(scores) @ topk @ WV[h]                     (WV folded after softmax)
  mhta   = sum_h ctx[h] @ G[h] + bias0                        G = WV[h] WO_h
  logit  = MLP(concat(mhta, tgt))

Sharding: pure data parallel, batch 2048 -> 8 cores x 256.

v2 vs v1:
  - The SWDGE gather ucode runs on ONE Q7 core pair selected by queue_num;
    4 queues exist.  Issuing the 16 per-chunk gathers round-robin on queues
    0-3 runs descriptor generation on all 4 pairs concurrently (the v1
    bottleneck: 16 x ~21us serial on one pair).
  - All heavy PE traffic is bf16: the gathered tile is cast once per chunk
    (ACT), pair transposes stream bf16, and the score/ctx/G matmuls use bf16
    stationaries (FWL fast-weight-load path) instead of multi-pass fp32.
"""

import sys

if "/opt/trn_rl_repo" not in sys.path:
    sys.path.insert(0, "/opt/trn_rl_repo")

import numpy as np

import ml_dtypes

import concourse.bass as bass
import concourse.tile as tile
import concourse.mybir as mybir
from concourse import bacc
from concourse.bass_utils import run_bass_kernel_spmd

F32 = mybir.dt.float32
BF16 = mybir.dt.bfloat16
I32 = mybir.dt.int32
I16 = mybir.dt.int16
AF = mybir.ActivationFunctionType

B, L, K, D, H = 2048, 1024, 128, 64, 4
N_CORES = 8
B_LOC = B // N_CORES  # 256
N_QUEUES = 4


def build(b_loc=B_LOC):
    """Build the per-core Bass module. b_loc must be a multiple of 32."""
    assert b_loc % 32 == 0
    n_grp = b_loc // 32            # gather groups of 32 samples
    n_pair = b_loc // 2            # sample pairs
    n_bt = (b_loc + 127) // 128    # 128-row tiles over the local batch

    nc = bacc.Bacc("TRN2", target_bir_lowering=False, debug=False,
                   num_devices=N_CORES, num_swdge_queues=N_QUEUES)

    useq = nc.dram_tensor("useq", [b_loc * L, D], F32, kind="ExternalInput").ap()
    tgt = nc.dram_tensor("tgt", [b_loc, D], F32, kind="ExternalInput").ap()
    wq = nc.dram_tensor("wq", [H * D, D], F32, kind="ExternalInput").ap()
    wk = nc.dram_tensor("wk", [H * D, D], F32, kind="ExternalInput").ap()
    wv = nc.dram_tensor("wv", [H * D, D], F32, kind="ExternalInput").ap()
    bq = nc.dram_tensor("bq", [H, D], F32, kind="ExternalInput").ap()
    bv = nc.dram_tensor("bv", [H, D], F32, kind="ExternalInput").ap()
    wo = nc.dram_tensor("wo", [H * D, D], F32, kind="ExternalInput").ap()
    bo = nc.dram_tensor("bo", [D, 1], F32, kind="ExternalInput").ap()
    w1 = nc.dram_tensor("w1", [2 * D, D], F32, kind="ExternalInput").ap()
    b1 = nc.dram_tensor("b1", [D, 1], F32, kind="ExternalInput").ap()
    w2 = nc.dram_tensor("w2", [D, 1], F32, kind="ExternalInput").ap()
    b2 = nc.dram_tensor("b2", [1, 1], F32, kind="ExternalInput").ap()
    # host-precomputed wrapped gather-index table (int16, replicated x8 over
    # partitions) and identity matrices
    widx = nc.dram_tensor("widx", [128, 8 * b_loc], I16, kind="ExternalInput").ap()
    ident_d = nc.dram_tensor("ident", [128, 128], F32, kind="ExternalInput").ap()
    ident16_d = nc.dram_tensor("ident16", [128, 128], BF16, kind="ExternalInput").ap()
    logit = nc.dram_tensor("logit", [b_loc, 1], F32, kind="ExternalOutput").ap()

    with tile.TileContext(nc) as tc, \
         tc.tile_pool(name="const", bufs=1) as const, \
         tc.tile_pool(name="stage", bufs=3) as stage, \
         tc.tile_pool(name="gath", bufs=6) as gath_pool, \
         tc.tile_pool(name="g16", bufs=4) as g16_pool, \
         tc.tile_pool(name="ptsb", bufs=6) as pt_pool, \
         tc.tile_pool(name="small", bufs=2) as small, \
         tc.tile_pool(name="expsb", bufs=2) as exp_pool, \
         tc.tile_pool(name="ps_a", bufs=1, space="PSUM") as ps_a, \
         tc.tile_pool(name="ptps", bufs=2, space="PSUM") as ptps_pool, \
         tc.tile_pool(name="scx", bufs=2, space="PSUM") as scx_pool, \
         tc.tile_pool(name="zc", bufs=1, space="PSUM") as zc_pool, \
         tc.tile_pool(name="mhps", bufs=1, space="PSUM") as mh_pool:

        # ---- gather index table (host-precomputed; one load, gathers first) ----
        # Gather order i = 32*128*g + 128*s + j; the SWDGE ucode reads idx i at
        # idxs[i % 16, i // 16] (int16), replicated into all 8 16-partition
        # blocks (one per Q7 core). Values are row offsets local to the group's
        # 32-sample window of useq: idx[s,j] + 1024*(s%32). Chunk (g, half)
        # reads the contiguous column slice [256g+128*half, +128).
        widx_sb = const.tile([128, 8 * b_loc], I16, tag="widx")
        nc.sync.dma_start(out=widx_sb[:], in_=widx[:])

        ident = const.tile([128, 128], F32, tag="ident")
        nc.sync.dma_start(out=ident[:], in_=ident_d[:])
        ident16 = const.tile([128, 128], BF16, tag="ident16")
        nc.sync.dma_start(out=ident16[:], in_=ident16_d[:])
        ones_col = const.tile([128, 1], BF16, tag="ones_col")
        nc.vector.memset(ones_col[:], 1.0)
        ones_row = const.tile([1, 128], F32, tag="ones_row")
        nc.vector.memset(ones_row[:], 1.0)

        # ---- weight transposes: wqT/wkT/wvT [64, 256] = [e, (h, d)] ----
        wT = {}
        for name, dram in (("wq", wq), ("wk", wk), ("wv", wv)):
            t_sb = const.tile([D, H * D], F32, tag=f"{name}T")
            for t in range(2):
                s = stage.tile([128, D], F32, tag="wstage")
                nc.sync.dma_start(out=s[:], in_=dram[128 * t:128 * (t + 1), :])
                tr = ps_a.tile([D, 128], F32, tag="pa")
                nc.tensor.transpose(tr[:], s[:], ident[:])
                nc.scalar.activation(t_sb[:, 128 * t:128 * (t + 1)], tr[:], AF.Copy)
            wT[name] = t_sb

        # ---- wo_r [64, 256] = [e, (h, f)] (reshaped, not transposed) ----
        wo_r = const.tile([D, H * D], F32, tag="wo_r")
        nc.sync.dma_start(out=wo_r[:].rearrange("p (h f) -> p h f", h=H),
                          in_=wo[:].rearrange("(h e) f -> e h f", h=H))

        # ---- small bias columns ----
        bqcol = const.tile([D, H], F32, tag="bqcol")
        nc.sync.dma_start(out=bqcol[:], in_=bq[:].rearrange("h e -> e h"))
        bvcol = const.tile([D, H], F32, tag="bvcol")
        nc.sync.dma_start(out=bvcol[:], in_=bv[:].rearrange("h e -> e h"))
        bocol = const.tile([D, 1], F32, tag="bocol")
        nc.sync.dma_start(out=bocol[:], in_=bo[:])
        b1col = const.tile([D, 1], F32, tag="b1col")
        nc.sync.dma_start(out=b1col[:], in_=b1[:])
        w1_sb = const.tile([2 * D, D], F32, tag="w1")
        nc.sync.dma_start(out=w1_sb[:], in_=w1[:])
        w2_sb = const.tile([D, 1], F32, tag="w2")
        nc.sync.dma_start(out=w2_sb[:], in_=w2[:])
        b2_sb = const.tile([1, 1], F32, tag="b2")
        nc.sync.dma_start(out=b2_sb[:], in_=b2[:])

        # ---- target transpose: tgtT [64, b_loc]; also xT rows 64:128 ----
        tgtT = const.tile([D, b_loc], F32, tag="tgtT")
        xT = const.tile([128, b_loc], F32, tag="xT")
        for t in range(n_bt):
            r0, r1 = 128 * t, min(128 * (t + 1), b_loc)
            n = r1 - r0
            s = stage.tile([128, D], F32, tag="tstage")
            nc.sync.dma_start(out=s[:n, :], in_=tgt[r0:r1, :])
            tr = ps_a.tile([D, 128], F32, tag="pa")
            nc.tensor.transpose(tr[:, :n], s[:n, :], ident[:n, :n])
            nc.scalar.activation(tgtT[:, r0:r1], tr[:, :n], AF.Copy)
            nc.scalar.activation(xT[D:2 * D, r0:r1], tr[:, :n], AF.Copy)

        # ---- per-head folded matrices ----
        # A_sb[:, 64h:64h+64] = (WQ[h] @ WK[h]^T) / 8 ; c_col = (WK[h] @ bQ[h]) / 8
        A_sb = const.tile([D, H * D], F32, tag="A")
        c_col = const.tile([128, H], F32, tag="c_col")
        c_ps = ps_a.tile([128, H], F32, tag="pa")
        for h in range(H):
            a_ps = ps_a.tile([D, D], F32, tag="pa")
            nc.tensor.matmul(a_ps[:], lhsT=wT["wq"][:, D * h:D * (h + 1)],
                             rhs=wT["wk"][:, D * h:D * (h + 1)], start=True, stop=True)
            nc.scalar.activation(A_sb[:, D * h:D * (h + 1)], a_ps[:], AF.Copy,
                                 scale=0.125)
            nc.tensor.matmul(c_ps[0:D, h:h + 1], lhsT=wT["wk"][:, D * h:D * (h + 1)],
                             rhs=bqcol[:, h:h + 1], start=True, stop=True)
            nc.tensor.matmul(c_ps[D:2 * D, h:h + 1], lhsT=wT["wk"][:, D * h:D * (h + 1)],
                             rhs=bqcol[:, h:h + 1], start=True, stop=True)
        nc.scalar.activation(c_col[:], c_ps[:], AF.Copy, scale=0.125)

        # ---- qk block-diagonal tile [128, 8*n_pair] (bf16) ----
        # pair q columns 8q..8q+7: cols 0-3 = even sample heads (rows 0:64),
        # cols 4-7 = odd sample heads (rows 64:128); rest zero.
        qk_bd = const.tile([128, 8 * n_pair], BF16, tag="qk_bd")
        nc.vector.memset(qk_bd[:], 0.0)
        tgtT_v = tgtT[:].rearrange("p (s two) -> p s two", two=2)
        qk_v = qk_bd[:].rearrange("p (q c) -> p q c", c=8)
        for h in range(H):
            qk_ps = ps_a.tile([128, n_pair], F32, tag="pa")
            nc.tensor.matmul(qk_ps[0:D, :], lhsT=A_sb[:, D * h:D * (h + 1)],
                             rhs=tgtT_v[:, :, 0], start=True, stop=True)
            nc.tensor.matmul(qk_ps[D:2 * D, :], lhsT=A_sb[:, D * h:D * (h + 1)],
                             rhs=tgtT_v[:, :, 1], start=True, stop=True)
            nc.scalar.activation(qk_v[0:D, :, h], qk_ps[0:D, :], AF.Identity,
                                 bias=c_col[0:D, h:h + 1], scale=1.0)
            nc.scalar.activation(qk_v[D:2 * D, :, 4 + h], qk_ps[D:2 * D, :], AF.Identity,
                                 bias=c_col[D:2 * D, h:h + 1], scale=1.0)

        # ---- G_sb [128, 256] bf16: rows 0:64 and 64:128 both hold G[h] = WV[h] @ WO_h ----
        G_sb = const.tile([128, H * D], BF16, tag="G")
        for h in range(H):
            g_ps = ps_a.tile([128, D], F32, tag="pa")
            nc.tensor.matmul(g_ps[0:D, :], lhsT=wT["wv"][:, D * h:D * (h + 1)],
                             rhs=wo_r[:, D * h:D * (h + 1)], start=True, stop=True)
            nc.tensor.matmul(g_ps[D:2 * D, :], lhsT=wT["wv"][:, D * h:D * (h + 1)],
                             rhs=wo_r[:, D * h:D * (h + 1)], start=True, stop=True)
            nc.scalar.activation(G_sb[:, D * h:D * (h + 1)], g_ps[:], AF.Copy)

        # ---- bias0 = sum_h bV[h] @ WO_h + bO, as a [64, 1] column ----
        b0_ps = ps_a.tile([1, D], F32, tag="pa")
        for h in range(H):
            nc.tensor.matmul(b0_ps[:], lhsT=bvcol[:, h:h + 1],
                             rhs=wo_r[:, D * h:D * (h + 1)],
                             start=(h == 0), stop=(h == H - 1))
        b0row = stage.tile([1, D], F32, tag="b0row")
        nc.scalar.activation(b0row[:], b0_ps[:], AF.Copy)
        b0c_ps = ps_a.tile([D, 1], F32, tag="pa")
        nc.tensor.transpose(b0c_ps[:], b0row[:], ident[0:1, 0:1])
        bias0 = const.tile([D, 1], F32, tag="bias0")
        nc.vector.tensor_tensor(out=bias0[:], in0=b0c_ps[:], in1=bocol[:],
                                op=mybir.AluOpType.add)

        # ---- main loop over chunks of 16 samples ----
        n_chunk = b_loc // 16
        ctxn = const.tile([128, 2 * b_loc], BF16, tag="ctxn")
        ctxn_v = ctxn[:].rearrange("p (cc q c) -> p cc q c", cc=n_chunk, c=4)
        mh_e = mh_pool.tile([D, n_pair], F32, tag="mh_e")
        mh_o = mh_pool.tile([D, n_pair], F32, tag="mh_o")
        for c in range(n_chunk):
            g, half = c // 2, c % 2
            gth = gath_pool.tile([128, 16 * D], F32, tag="gath")
            idxs_c = widx_sb[:, 128 * c:128 * (c + 1)]
            nc.gpsimd.dma_gather(
                out_ap=gth[:].rearrange("p (s d) -> p s d", d=D),
                in_ap=useq[32 * L * g:32 * L * (g + 1), :],
                idxs_ap=idxs_c,
                num_idxs=16 * K, num_idxs_reg=16 * K, elem_size=D,
                single_packet=False, queue_num=c % N_QUEUES)
            g16 = g16_pool.tile([128, 16 * D], BF16, tag="g16")
            nc.scalar.activation(g16[:], gth[:], AF.Copy)
            sc_ps = scx_pool.tile([128, 64], F32, tag="scx")
            for q in range(8):
                tr_ps = ptps_pool.tile([128, 128], BF16, tag="trp")
                nc.tensor.transpose(tr_ps[:], g16[:, 128 * q:128 * (q + 1)],
                                    ident16[:])
                pT = pt_pool.tile([128, 128], BF16, tag="pt")
                if q % 2 == 0:
                    nc.vector.tensor_copy(out=pT[:], in_=tr_ps[:])
                else:
                    nc.scalar.activation(pT[:], tr_ps[:], AF.Copy)
                Q = 8 * c + q
                nc.tensor.matmul(sc_ps[:, 8 * q:8 * (q + 1)], lhsT=pT[:],
                                 rhs=qk_bd[:, 8 * Q:8 * (Q + 1)], start=True, stop=True)
            exp_sb = exp_pool.tile([128, 64], BF16, tag="exp")
            nc.scalar.activation(exp_sb[:], sc_ps[:], AF.Exp)
            # op2 on UNNORMALIZED exp (releases g16 asap); 1/Z folded in below.
            ctx_ps = scx_pool.tile([128, 64], F32, tag="scx")
            for q in range(8):
                nc.tensor.matmul(ctx_ps[:, 8 * q:8 * (q + 1)],
                                 lhsT=g16[:, 128 * q:128 * (q + 1)],
                                 rhs=exp_sb[:, 8 * q:8 * (q + 1)], start=True, stop=True)
            z_ps = zc_pool.tile([64, 1], F32, tag="zc")
            nc.tensor.matmul(z_ps[:], lhsT=exp_sb[:], rhs=ones_col[:],
                             start=True, stop=True)
            rz = small.tile([64, 1], F32, tag="rz")
            nc.vector.reciprocal(rz[:], z_ps[:])
            rzr_ps = zc_pool.tile([1, 64], F32, tag="zc")
            nc.tensor.transpose(rzr_ps[:], rz[:], ident[0:64, 0:64])
            rzr = small.tile([1, 64], F32, tag="rzr")
            nc.scalar.activation(rzr[:], rzr_ps[:], AF.Copy)
            rzb_ps = zc_pool.tile([128, 64], F32, tag="zc")
            nc.tensor.matmul(rzb_ps[:], lhsT=ones_row[:], rhs=rzr[:],
                             start=True, stop=True)
            rzb = exp_pool.tile([128, 64], F32, tag="rzb")
            nc.scalar.activation(rzb[:], rzb_ps[:], AF.Copy)
            ctx_v = ctx_ps[:].rearrange("p (q c) -> p q c", c=8)
            rzb_v = rzb[:].rearrange("p (q c) -> p q c", c=8)
            nc.vector.tensor_tensor(out=ctxn_v[0:D, c, :, :], in0=ctx_v[0:D, :, 0:4],
                                    in1=rzb_v[0:D, :, 0:4], op=mybir.AluOpType.mult)
            nc.vector.tensor_tensor(out=ctxn_v[D:2 * D, c, :, :],
                                    in0=ctx_v[D:2 * D, :, 4:8],
                                    in1=rzb_v[D:2 * D, :, 4:8],
                                    op=mybir.AluOpType.mult)
            for h in range(H):
                nc.tensor.matmul(mh_e[:, 8 * c:8 * (c + 1)],
                                 lhsT=G_sb[0:D, D * h:D * (h + 1)],
                                 rhs=ctxn_v[0:D, c, :, h],
                                 start=(h == 0), stop=(h == H - 1))
            for h in range(H):
                nc.tensor.matmul(mh_o[:, 8 * c:8 * (c + 1)],
                                 lhsT=G_sb[D:2 * D, D * h:D * (h + 1)],
                                 rhs=ctxn_v[D:2 * D, c, :, h],
                                 start=(h == 0), stop=(h == H - 1))

        # ---- output head tail: bias + MLP ----
        x_v = xT[:].rearrange("p (s two) -> p s two", two=2)
        nc.scalar.activation(x_v[0:D, :, 0], mh_e[:], AF.Identity, bias=bias0[:],
                             scale=1.0)
        nc.scalar.activation(x_v[0:D, :, 1], mh_o[:], AF.Identity, bias=bias0[:],
                             scale=1.0)
        h1_ps = ps_a.tile([D, b_loc], F32, tag="pa")
        nc.tensor.matmul(h1_ps[:], lhsT=w1_sb[:], rhs=xT[:], start=True, stop=True)
        h1_sb = const.tile([D, b_loc], F32, tag="h1")
        nc.scalar.activation(h1_sb[:], h1_ps[:], AF.Relu, bias=b1col[:], scale=1.0)
        lg_ps = ps_a.tile([1, b_loc], F32, tag="pa")
        nc.tensor.matmul(lg_ps[:], lhsT=w2_sb[:], rhs=h1_sb[:], start=True, stop=True)
        lg_sb = const.tile([1, b_loc], F32, tag="lg")
        nc.scalar.activation(lg_sb[:], lg_ps[:], AF.Identity, bias=b2_sb[:], scale=1.0)
        nc.sync.dma_start(out=logit[:], in_=lg_sb[:])

    nc.compile()
    return nc


def _widx_table(idx_core, b_loc):
    """Wrapped int16 gather-index table for one core, [128, 8*b_loc].

    Layout matches the SWDGE gather ucode: gather i of chunk c reads the
    int16 at [i % 16, 128*c + i // 16]; values are row offsets local to each
    32-sample useq window (idx + 1024*(s % 32)); replicated x8 on partitions.
    """
    n_grp = b_loc // 32
    idxv = idx_core.astype(np.int16).reshape(n_grp, 32, 8, 16)  # [g, s, cj, p]
    w = idxv + (L * np.arange(32, dtype=np.int16))[None, :, None, None]
    w16 = w.transpose(3, 0, 1, 2).reshape(16, 8 * b_loc)
    return np.ascontiguousarray(np.tile(w16, (8, 1)))


def make_in_maps(inputs, b_loc=B_LOC, n_cores=N_CORES):
    """Shard full inputs into per-core in_maps (data parallel over batch)."""
    idx = np.asarray(inputs["indices"])
    if idx.dtype != np.int32:
        idx = idx.astype(np.int32)
    useq = np.ascontiguousarray(np.asarray(inputs["user_seq_emb"], dtype=np.float32))
    tgt = np.ascontiguousarray(np.asarray(inputs["target_emb"], dtype=np.float32)[:, 0, :])
    shared = {
        "ident": np.eye(128, dtype=np.float32),
        "ident16": np.eye(128, dtype=ml_dtypes.bfloat16),
        "wq": np.ascontiguousarray(np.asarray(inputs["WQ"], np.float32).reshape(H * D, D)),
        "wk": np.ascontiguousarray(np.asarray(inputs["WK"], np.float32).reshape(H * D, D)),
        "wv": np.ascontiguousarray(np.asarray(inputs["WV"], np.float32).reshape(H * D, D)),
        "bq": np.ascontiguousarray(np.asarray(inputs["bQ"], np.float32)),
        "bv": np.ascontiguousarray(np.asarray(inputs["bV"], np.float32)),
        "wo": np.ascontiguousarray(np.asarray(inputs["WO"], np.float32)),
        "bo": np.asarray(inputs["bO"], np.float32).reshape(D, 1).copy(),
        "w1": np.ascontiguousarray(np.asarray(inputs["W1"], np.float32)),
        "b1": np.asarray(inputs["b1"], np.float32).reshape(D, 1).copy(),
        "w2": np.ascontiguousarray(np.asarray(inputs["W2"], np.float32)),
        "b2": np.asarray(inputs["b2"], np.float32).reshape(1, 1).copy(),
    }
    in_maps = []
    for c in range(n_cores):
        s = slice(c * b_loc, (c + 1) * b_loc)
        m = dict(shared)
        m["useq"] = useq[s].reshape(b_loc * L, D)
        m["tgt"] = tgt[s]
        m["widx"] = _widx_table(idx[s], b_loc)
        in_maps.append(m)
    return in_maps


_NC_CACHE = {}


def kernel(**inputs):
    if B_LOC not in _NC_CACHE:
        _NC_CACHE[B_LOC] = build(B_LOC)
    nc = _NC_CACHE[B_LOC]
    in_maps = make_in_maps(inputs)
    res = run_bass_kernel_spmd(nc, in_maps, core_ids=list(range(N_CORES)))
    return np.concatenate([res.results[c]["logit"] for c in range(N_CORES)], axis=0)


# revision 25
# speedup vs baseline: 2.7212x; 1.0113x over previous
"""Trainium2 Bass kernel for nn_BaseSearchBasedModel (sparse attention).

Math restructuring (exact up to rounding):
  topk   = user_seq_emb[b, indices[b,k]]                      (device gather)
  scores = topk . (A[h]^T tgt + c[h]) / 8  + const(b,h)       A = WQ WK^T, c = WK bQ
    The const(b,h) term (q.bK) is constant over the softmax axis -> drops out.
  heads  = softmax(scores) @ topk @ WV[h]                     (WV folded after softmax)
  mhta   = sum_h ctx[h] @ G[h] + bias0                        G = WV[h] WO_h
  logit  = MLP(concat(mhta, tgt))

Sharding: pure data parallel, batch 2048 -> 8 cores x 256.

v2 vs v1:
  - The SWDGE gather ucode runs on ONE Q7 core pair selected by queue_num;
    4 queues exist.  Issuing the 16 per-chunk gathers round-robin on queues
    0-3 runs descriptor generation on all 4 pairs concurrently (the v1
    bottleneck: 16 x ~21us serial on one pair).
  - All heavy PE traffic is bf16: the gathered tile is cast once per chunk
    (ACT), pair transposes stream bf16, and the score/ctx/G matmuls use bf16
    stationaries (FWL fast-weight-load path) instead of multi-pass fp32.
"""

import sys

if "/opt/trn_rl_repo" not in sys.path:
    sys.path.insert(0, "/opt/trn_rl_repo")

import numpy as np

import ml_dtypes

import concourse.bass as bass
import concourse.tile as tile
import concourse.mybir as mybir
from concourse import bacc
from concourse.bass_utils import run_bass_kernel_spmd

F32 = mybir.dt.float32
BF16 = mybir.dt.bfloat16
I32 = mybir.dt.int32
I16 = mybir.dt.int16
AF = mybir.ActivationFunctionType

B, L, K, D, H = 2048, 1024, 128, 64, 4
N_CORES = 8
B_LOC = B // N_CORES  # 256
N_QUEUES = 4


def build(b_loc=B_LOC):
    """Build the per-core Bass module. b_loc must be a multiple of 32."""
    assert b_loc % 32 == 0
    n_grp = b_loc // 32            # gather groups of 32 samples
    n_pair = b_loc // 2            # sample pairs
    n_bt = (b_loc + 127) // 128    # 128-row tiles over the local batch

    nc = bacc.Bacc("TRN2", target_bir_lowering=False, debug=False,
                   num_devices=N_CORES, num_swdge_queues=N_QUEUES)

    useq = nc.dram_tensor("useq", [b_loc * L, D], F32, kind="ExternalInput").ap()
    tgt = nc.dram_tensor("tgt", [b_loc, D], F32, kind="ExternalInput").ap()
    wq = nc.dram_tensor("wq", [H * D, D], F32, kind="ExternalInput").ap()
    wk = nc.dram_tensor("wk", [H * D, D], F32, kind="ExternalInput").ap()
    wv = nc.dram_tensor("wv", [H * D, D], F32, kind="ExternalInput").ap()
    bq = nc.dram_tensor("bq", [H, D], F32, kind="ExternalInput").ap()
    bv = nc.dram_tensor("bv", [H, D], F32, kind="ExternalInput").ap()
    wo = nc.dram_tensor("wo", [H * D, D], F32, kind="ExternalInput").ap()
    bo = nc.dram_tensor("bo", [D, 1], F32, kind="ExternalInput").ap()
    w1 = nc.dram_tensor("w1", [2 * D, D], F32, kind="ExternalInput").ap()
    b1 = nc.dram_tensor("b1", [D, 1], F32, kind="ExternalInput").ap()
    w2 = nc.dram_tensor("w2", [D, 1], F32, kind="ExternalInput").ap()
    b2 = nc.dram_tensor("b2", [1, 1], F32, kind="ExternalInput").ap()
    # host-precomputed wrapped gather-index table (int16, replicated x8 over
    # partitions; chunk-0 slice split out so gathering starts immediately)
    # and identity matrices
    widx0 = nc.dram_tensor("widx0", [128, 128], I16, kind="ExternalInput").ap()
    widxr = nc.dram_tensor("widxr", [128, 8 * b_loc - 128], I16,
                           kind="ExternalInput").ap()
    ident_d = nc.dram_tensor("ident", [128, 128], F32, kind="ExternalInput").ap()
    ident16_d = nc.dram_tensor("ident16", [128, 128], BF16, kind="ExternalInput").ap()
    logit = nc.dram_tensor("logit", [b_loc, 1], F32, kind="ExternalOutput").ap()

    with tile.TileContext(nc) as tc, \
         tc.tile_pool(name="const", bufs=1) as const, \
         tc.tile_pool(name="stage", bufs=3) as stage, \
         tc.tile_pool(name="gath", bufs=6) as gath_pool, \
         tc.tile_pool(name="g16", bufs=4) as g16_pool, \
         tc.tile_pool(name="ptsb", bufs=12) as pt_pool, \
         tc.tile_pool(name="small", bufs=2) as small, \
         tc.tile_pool(name="expsb", bufs=2) as exp_pool, \
         tc.tile_pool(name="ps_a", bufs=1, space="PSUM") as ps_a, \
         tc.tile_pool(name="ptps", bufs=2, space="PSUM") as ptps_pool, \
         tc.tile_pool(name="scx", bufs=3, space="PSUM") as scx_pool, \
         tc.tile_pool(name="zc", bufs=1, space="PSUM") as zc_pool, \
         tc.tile_pool(name="mhps", bufs=1, space="PSUM") as mh_pool:

        # ---- gather index table (host-precomputed; loads first) ----
        # Gather order i = 32*128*g + 128*s + j; the SWDGE ucode reads idx i at
        # idxs[i % 16, i // 16] (int16), replicated into all 8 16-partition
        # blocks (one per Q7 core). Values are row offsets local to the group's
        # 32-sample window of useq: idx[s,j] + 1024*(s%32). Chunk (g, half)
        # owns the column slice [256g+128*half, +128); sub-gather j (4 samples,
        # queue j) owns cols [.. + 32j, +32). Chunk 0 comes from widx0.
        widx0_sb = const.tile([128, 128], I16, tag="widx0")
        nc.sync.dma_start(out=widx0_sb[:], in_=widx0[:])
        widx_sb = const.tile([128, 8 * b_loc - 128], I16, tag="widx")
        nc.sync.dma_start(out=widx_sb[:], in_=widxr[:])

        ident = const.tile([128, 128], F32, tag="ident")
        nc.sync.dma_start(out=ident[:], in_=ident_d[:])
        ident16 = const.tile([128, 128], BF16, tag="ident16")
        nc.sync.dma_start(out=ident16[:], in_=ident16_d[:])
        ones_col = const.tile([128, 1], BF16, tag="ones_col")
        nc.vector.memset(ones_col[:], 1.0)
        ones_row = const.tile([1, 128], F32, tag="ones_row")
        nc.vector.memset(ones_row[:], 1.0)

        # ---- weight transposes: wqT/wkT/wvT [64, 256] = [e, (h, d)] ----
        wT = {}
        for name, dram in (("wq", wq), ("wk", wk), ("wv", wv)):
            t_sb = const.tile([D, H * D], F32, tag=f"{name}T")
            for t in range(2):
                s = stage.tile([128, D], F32, tag="wstage")
                nc.sync.dma_start(out=s[:], in_=dram[128 * t:128 * (t + 1), :])
                tr = ps_a.tile([D, 128], F32, tag="pa")
                nc.tensor.transpose(tr[:], s[:], ident[:])
                nc.scalar.activation(t_sb[:, 128 * t:128 * (t + 1)], tr[:], AF.Copy)
            wT[name] = t_sb

        # ---- wo_r [64, 256] = [e, (h, f)] (reshaped, not transposed) ----
        wo_r = const.tile([D, H * D], F32, tag="wo_r")
        nc.sync.dma_start(out=wo_r[:].rearrange("p (h f) -> p h f", h=H),
                          in_=wo[:].rearrange("(h e) f -> e h f", h=H))

        # ---- small bias columns ----
        bqcol = const.tile([D, H], F32, tag="bqcol")
        nc.sync.dma_start(out=bqcol[:], in_=bq[:].rearrange("h e -> e h"))
        bvcol = const.tile([D, H], F32, tag="bvcol")
        nc.sync.dma_start(out=bvcol[:], in_=bv[:].rearrange("h e -> e h"))
        bocol = const.tile([D, 1], F32, tag="bocol")
        nc.sync.dma_start(out=bocol[:], in_=bo[:])
        b1col = const.tile([D, 1], F32, tag="b1col")
        nc.sync.dma_start(out=b1col[:], in_=b1[:])
        w1_sb = const.tile([2 * D, D], F32, tag="w1")
        nc.sync.dma_start(out=w1_sb[:], in_=w1[:])
        w2_sb = const.tile([D, 1], F32, tag="w2")
        nc.sync.dma_start(out=w2_sb[:], in_=w2[:])
        b2_sb = const.tile([1, 1], F32, tag="b2")
        nc.sync.dma_start(out=b2_sb[:], in_=b2[:])

        # ---- target transpose: tgtT [64, b_loc]; also xT rows 64:128 ----
        tgtT = const.tile([D, b_loc], F32, tag="tgtT")
        xT = const.tile([128, b_loc], F32, tag="xT")
        for t in range(n_bt):
            r0, r1 = 128 * t, min(128 * (t + 1), b_loc)
            n = r1 - r0
            s = stage.tile([128, D], F32, tag="tstage")
            nc.sync.dma_start(out=s[:n, :], in_=tgt[r0:r1, :])
            tr = ps_a.tile([D, 128], F32, tag="pa")
            nc.tensor.transpose(tr[:, :n], s[:n, :], ident[:n, :n])
            nc.scalar.activation(tgtT[:, r0:r1], tr[:, :n], AF.Copy)
            nc.scalar.activation(xT[D:2 * D, r0:r1], tr[:, :n], AF.Copy)

        # ---- per-head folded matrices ----
        # A_sb[:, 64h:64h+64] = (WQ[h] @ WK[h]^T) / 8 ; c_col = (WK[h] @ bQ[h]) / 8
        A_sb = const.tile([D, H * D], F32, tag="A")
        c_col = const.tile([128, H], F32, tag="c_col")
        c_ps = ps_a.tile([128, H], F32, tag="pa")
        for h in range(H):
            a_ps = ps_a.tile([D, D], F32, tag="pa")
            nc.tensor.matmul(a_ps[:], lhsT=wT["wq"][:, D * h:D * (h + 1)],
                             rhs=wT["wk"][:, D * h:D * (h + 1)], start=True, stop=True)
            nc.scalar.activation(A_sb[:, D * h:D * (h + 1)], a_ps[:], AF.Copy,
                                 scale=0.125)
            nc.tensor.matmul(c_ps[0:D, h:h + 1], lhsT=wT["wk"][:, D * h:D * (h + 1)],
                             rhs=bqcol[:, h:h + 1], start=True, stop=True)
            nc.tensor.matmul(c_ps[D:2 * D, h:h + 1], lhsT=wT["wk"][:, D * h:D * (h + 1)],
                             rhs=bqcol[:, h:h + 1], start=True, stop=True)
        nc.scalar.activation(c_col[:], c_ps[:], AF.Copy, scale=0.125)

        # ---- qk block-diagonal tile [128, 8*n_pair] (bf16) ----
        # pair q columns 8q..8q+7: cols 0-3 = even sample heads (rows 0:64),
        # cols 4-7 = odd sample heads (rows 64:128); rest zero.
        qk_bd = const.tile([128, 8 * n_pair], BF16, tag="qk_bd")
        nc.vector.memset(qk_bd[:], 0.0)
        tgtT_v = tgtT[:].rearrange("p (s two) -> p s two", two=2)
        qk_v = qk_bd[:].rearrange("p (q c) -> p q c", c=8)
        for h in range(H):
            qk_ps = ps_a.tile([128, n_pair], F32, tag="pa")
            nc.tensor.matmul(qk_ps[0:D, :], lhsT=A_sb[:, D * h:D * (h + 1)],
                             rhs=tgtT_v[:, :, 0], start=True, stop=True)
            nc.tensor.matmul(qk_ps[D:2 * D, :], lhsT=A_sb[:, D * h:D * (h + 1)],
                             rhs=tgtT_v[:, :, 1], start=True, stop=True)
            nc.scalar.activation(qk_v[0:D, :, h], qk_ps[0:D, :], AF.Identity,
                                 bias=c_col[0:D, h:h + 1], scale=1.0)
            nc.scalar.activation(qk_v[D:2 * D, :, 4 + h], qk_ps[D:2 * D, :], AF.Identity,
                                 bias=c_col[D:2 * D, h:h + 1], scale=1.0)

        # ---- G_sb [128, 256] bf16: rows 0:64 and 64:128 both hold G[h] = WV[h] @ WO_h ----
        G_sb = const.tile([128, H * D], BF16, tag="G")
        for h in range(H):
            g_ps = ps_a.tile([128, D], F32, tag="pa")
            nc.tensor.matmul(g_ps[0:D, :], lhsT=wT["wv"][:, D * h:D * (h + 1)],
                             rhs=wo_r[:, D * h:D * (h + 1)], start=True, stop=True)
            nc.tensor.matmul(g_ps[D:2 * D, :], lhsT=wT["wv"][:, D * h:D * (h + 1)],
                             rhs=wo_r[:, D * h:D * (h + 1)], start=True, stop=True)
            nc.scalar.activation(G_sb[:, D * h:D * (h + 1)], g_ps[:], AF.Copy)

        # ---- bias0 = sum_h bV[h] @ WO_h + bO, as a [64, 1] column ----
        b0_ps = ps_a.tile([1, D], F32, tag="pa")
        for h in range(H):
            nc.tensor.matmul(b0_ps[:], lhsT=bvcol[:, h:h + 1],
                             rhs=wo_r[:, D * h:D * (h + 1)],
                             start=(h == 0), stop=(h == H - 1))
        b0row = stage.tile([1, D], F32, tag="b0row")
        nc.scalar.activation(b0row[:], b0_ps[:], AF.Copy)
        b0c_ps = ps_a.tile([D, 1], F32, tag="pa")
        nc.tensor.transpose(b0c_ps[:], b0row[:], ident[0:1, 0:1])
        bias0 = const.tile([D, 1], F32, tag="bias0")
        nc.vector.tensor_tensor(out=bias0[:], in0=b0c_ps[:], in1=bocol[:],
                                op=mybir.AluOpType.add)

        # ---- main loop over chunks of 16 samples ----
        n_chunk = b_loc // 16
        ctxn = const.tile([128, 2 * b_loc], BF16, tag="ctxn")
        ctxn_v = ctxn[:].rearrange("p (cc q c) -> p cc q c", cc=n_chunk, c=4)
        # mh_e = cols 0:n_pair, mh_o = cols n_pair:2*n_pair (single PSUM bank)
        mh = mh_pool.tile([D, 2 * n_pair], F32, tag="mh")

        def mlp_half(t):
            """MLP tail for samples [128t, 128(t+1)) once their mh cols exist."""
            s0 = 64 * t
            x_v = xT[:].rearrange("p (s two) -> p s two", two=2)
            nc.scalar.activation(x_v[0:D, 64 * t:64 * (t + 1), 0],
                                 mh[:, s0:s0 + 64], AF.Identity,
                                 bias=bias0[:], scale=1.0)
            nc.scalar.activation(x_v[0:D, 64 * t:64 * (t + 1), 1],
                                 mh[:, n_pair + s0:n_pair + s0 + 64], AF.Identity,
                                 bias=bias0[:], scale=1.0)
            h1_ps = ps_a.tile([D, 128], F32, tag="pa")
            nc.tensor.matmul(h1_ps[:], lhsT=w1_sb[:], rhs=xT[:, 128 * t:128 * (t + 1)],
                             start=True, stop=True)
            h1_sb = stage.tile([D, 128], F32, tag="h1")
            nc.scalar.activation(h1_sb[:], h1_ps[:], AF.Relu, bias=b1col[:], scale=1.0)
            lg_ps = ps_a.tile([1, 128], F32, tag="pa")
            nc.tensor.matmul(lg_ps[:], lhsT=w2_sb[:], rhs=h1_sb[:], start=True, stop=True)
            lg_sb = stage.tile([1, 128], F32, tag="lg")
            nc.scalar.activation(lg_sb[:], lg_ps[:], AF.Identity, bias=b2_sb[:],
                                 scale=1.0)
            nc.sync.dma_start(out=logit[128 * t:128 * (t + 1), :], in_=lg_sb[:])

        for c in range(n_chunk):
            g, half = c // 2, c % 2
            gth = gath_pool.tile([128, 16 * D], F32, tag="gath")
            gth_v = gth[:].rearrange("p (s d) -> p s d", d=D)
            idxs_c = (widx0_sb[:] if c == 0 else
                      widx_sb[:, 128 * (c - 1):128 * c])
            nc.gpsimd.dma_gather(
                out_ap=gth_v[:],
                in_ap=useq[32 * L * g:32 * L * (g + 1), :],
                idxs_ap=idxs_c,
                num_idxs=16 * K, num_idxs_reg=16 * K, elem_size=D,
                single_packet=False, queue_num=c % N_QUEUES)
            g16 = g16_pool.tile([128, 16 * D], BF16, tag="g16")
            nc.scalar.activation(g16[:, 0:8 * D], gth[:, 0:8 * D], AF.Copy)
            nc.scalar.activation(g16[:, 8 * D:16 * D], gth[:, 8 * D:16 * D], AF.Copy)
            # phase 1: pair transposes (PE) + copies (DVE). Each transpose gets
            # its own PSUM bank buffer: PE-write + DVE-read of the SAME bank is
            # a fatal HW collision, so tr tiles must rotate across banks.
            pTs = []
            for q in range(8):
                tr_ps = ptps_pool.tile([128, 128], BF16, tag="trp")
                nc.tensor.transpose(tr_ps[:],
                                    g16[:, 128 * q:128 * (q + 1)], ident16[:])
                pT = pt_pool.tile([128, 128], BF16, tag="pt")
                nc.vector.tensor_copy(out=pT[:], in_=tr_ps[:])
                pTs.append(pT)
            # phase 2: 8 score matmuls back-to-back
            sc_ps = scx_pool.tile([128, 64], F32, tag="scx")
            for q in range(8):
                Q = 8 * c + q
                nc.tensor.matmul(sc_ps[:, 8 * q:8 * (q + 1)], lhsT=pTs[q][:],
                                 rhs=qk_bd[:, 8 * Q:8 * (Q + 1)], start=True, stop=True)
            exp_sb = exp_pool.tile([128, 64], BF16, tag="exp")
            nc.scalar.activation(exp_sb[:], sc_ps[:], AF.Exp)
            # phase 3: ctx on UNNORMALIZED exp (1/Z folded in below) + row-Z
            ctx_ps = scx_pool.tile([128, 64], F32, tag="scx")
            for q in range(8):
                nc.tensor.matmul(ctx_ps[:, 8 * q:8 * (q + 1)],
                                 lhsT=g16[:, 128 * q:128 * (q + 1)],
                                 rhs=exp_sb[:, 8 * q:8 * (q + 1)], start=True, stop=True)
            z_ps = zc_pool.tile([64, 1], F32, tag="zc")
            nc.tensor.matmul(z_ps[:], lhsT=exp_sb[:], rhs=ones_col[:],
                             start=True, stop=True)
            rz = small.tile([64, 1], F32, tag="rz")
            nc.vector.reciprocal(rz[:], z_ps[:])
            rzr_ps = zc_pool.tile([1, 64], F32, tag="zc")
            nc.tensor.transpose(rzr_ps[:], rz[:], ident[0:64, 0:64])
            rzr = small.tile([1, 64], F32, tag="rzr")
            nc.scalar.activation(rzr[:], rzr_ps[:], AF.Copy)
            rzb_ps = zc_pool.tile([128, 64], F32, tag="zc")
            nc.tensor.matmul(rzb_ps[:], lhsT=ones_row[:], rhs=rzr[:],
                             start=True, stop=True)
            rzb = exp_pool.tile([128, 64], F32, tag="rzb")
            nc.vector.tensor_copy(out=rzb[:], in_=rzb_ps[:])
            ctx_v = ctx_ps[:].rearrange("p (q c) -> p q c", c=8)
            rzb_v = rzb[:].rearrange("p (q c) -> p q c", c=8)
            nc.vector.tensor_tensor(out=ctxn_v[0:D, c, :, :], in0=ctx_v[0:D, :, 0:4],
                                    in1=rzb_v[0:D, :, 0:4], op=mybir.AluOpType.mult)
            nc.vector.tensor_tensor(out=ctxn_v[D:2 * D, c, :, :],
                                    in0=ctx_v[D:2 * D, :, 4:8],
                                    in1=rzb_v[D:2 * D, :, 4:8],
                                    op=mybir.AluOpType.mult)
            for h in range(H):
                nc.tensor.matmul(mh[:, 8 * c:8 * (c + 1)],
                                 lhsT=G_sb[0:D, D * h:D * (h + 1)],
                                 rhs=ctxn_v[0:D, c, :, h],
                                 start=(h == 0), stop=(h == H - 1))
            for h in range(H):
                nc.tensor.matmul(mh[:, n_pair + 8 * c:n_pair + 8 * (c + 1)],
                                 lhsT=G_sb[D:2 * D, D * h:D * (h + 1)],
                                 rhs=ctxn_v[D:2 * D, c, :, h],
                                 start=(h == 0), stop=(h == H - 1))
        # MLP tail only after ALL G-matmuls: reading the mh PSUM bank while PE
        # still accumulates into it is a fatal HW bank collision.
        mlp_half(0)
        mlp_half(1)

    nc.compile()
    return nc


def _widx_table(idx_core, b_loc):
    """Wrapped int16 gather-index table for one core, [128, 8*b_loc].

    Layout matches the SWDGE gather ucode: gather i of chunk c reads the
    int16 at [i % 16, 128*c + i // 16]; values are row offsets local to each
    32-sample useq window (idx + 1024*(s % 32)); replicated x8 on partitions.
    """
    n_grp = b_loc // 32
    idxv = idx_core.astype(np.int16).reshape(n_grp, 32, 8, 16)  # [g, s, cj, p]
    w = idxv + (L * np.arange(32, dtype=np.int16))[None, :, None, None]
    w16 = w.transpose(3, 0, 1, 2).reshape(16, 8 * b_loc)
    return np.ascontiguousarray(np.tile(w16, (8, 1)))


def make_in_maps(inputs, b_loc=B_LOC, n_cores=N_CORES):
    """Shard full inputs into per-core in_maps (data parallel over batch)."""
    idx = np.asarray(inputs["indices"])
    if idx.dtype != np.int32:
        idx = idx.astype(np.int32)
    useq = np.ascontiguousarray(np.asarray(inputs["user_seq_emb"], dtype=np.float32))
    tgt = np.ascontiguousarray(np.asarray(inputs["target_emb"], dtype=np.float32)[:, 0, :])
    shared = {
        "ident": np.eye(128, dtype=np.float32),
        "ident16": np.eye(128, dtype=ml_dtypes.bfloat16),
        "wq": np.ascontiguousarray(np.asarray(inputs["WQ"], np.float32).reshape(H * D, D)),
        "wk": np.ascontiguousarray(np.asarray(inputs["WK"], np.float32).reshape(H * D, D)),
        "wv": np.ascontiguousarray(np.asarray(inputs["WV"], np.float32).reshape(H * D, D)),
        "bq": np.ascontiguousarray(np.asarray(inputs["bQ"], np.float32)),
        "bv": np.ascontiguousarray(np.asarray(inputs["bV"], np.float32)),
        "wo": np.ascontiguousarray(np.asarray(inputs["WO"], np.float32)),
        "bo": np.asarray(inputs["bO"], np.float32).reshape(D, 1).copy(),
        "w1": np.ascontiguousarray(np.asarray(inputs["W1"], np.float32)),
        "b1": np.asarray(inputs["b1"], np.float32).reshape(D, 1).copy(),
        "w2": np.ascontiguousarray(np.asarray(inputs["W2"], np.float32)),
        "b2": np.asarray(inputs["b2"], np.float32).reshape(1, 1).copy(),
    }
    in_maps = []
    for c in range(n_cores):
        s = slice(c * b_loc, (c + 1) * b_loc)
        m = dict(shared)
        m["useq"] = useq[s].reshape(b_loc * L, D)
        m["tgt"] = tgt[s]
        w = _widx_table(idx[s], b_loc)
        m["widx0"] = np.ascontiguousarray(w[:, :128])
        m["widxr"] = np.ascontiguousarray(w[:, 128:])
        in_maps.append(m)
    return in_maps


_NC_CACHE = {}


def kernel(**inputs):
    if B_LOC not in _NC_CACHE:
        _NC_CACHE[B_LOC] = build(B_LOC)
    nc = _NC_CACHE[B_LOC]
    in_maps = make_in_maps(inputs)
    res = run_bass_kernel_spmd(nc, in_maps, core_ids=list(range(N_CORES)))
    return np.concatenate([res.results[c]["logit"] for c in range(N_CORES)], axis=0)
